# revision 1
# baseline (speedup 1.0000x reference)
"""Trainium2 Bass kernel for nn_CrossCorrV2.

Math: with P = nd0*nd1 = 100 patches and OUT_CHANNEL = 100, top_k over the
patch axis returns *all* patches, so mean(top_k) == mean over patches.  Both
the grouped conv (linear in the filter) and the bilinear resize (linear map)
commute with that mean, so the whole module collapses per sample to

    out[b] = resize129->128( corr2d(x1n[b], K[b]) ) / (hh*ww*P)

where x1n/x2n are channel-L2-normalized and K[b][c, dy, dx] =
sum_{grid} x2n[b, c, i0*6+dy, i1*6+dx] (a 6x6 fold of the normalized x2).

Per-core pipeline (1 sample per NeuronCore, 8 cores), instruction-count
optimized (this environment executes NEFFs at a fixed ~25-40us per
instruction, so fewer/bigger ops win; the layout is also close to the
real-HW-optimal structure):

  - x2 ([c, pix] layout): square -> ones-matmul sumsq -> sqrt/recip on a
    [60,60] reshape -> K=1-matmul partition broadcast -> normalize ->
    two nested-AP tensor_reduce folds -> Kb [64, 48] (zero-padded to 48).
  - x1 sumsq: square (split ACT/DVE over the two 64-row half tiles) ->
    ones-matmuls -> flat [1,16384] -> DMA reshape [r,j] -> sqrt/recip ->
    PE transpose -> nrmT [j, r].
  - conv: stationary Kb loaded ONCE, 32 matmuls streaming x1 [64, 512]
    chunks -> Z [48, 16384] PSUM -> copy-cast bf16 -> 4 dma_start_transpose
    (xbar) -> ZT [j, o, r] -> ONE broadcast multiply by nrmT -> ZTs f32.
  - col2im: 6 shifted adds (dy) -> Ydx[j, dx, Ipad]; 12 PE transposes
    accumulate into PSUM YF[I, J'] at per-dx free offsets (boundary
    handling comes free from the written windows).
  - separable resize (weights replicate jax.image.resize bilinear with
    antialias), final PE transpose, out as [J, I]; host transposes back.
"""

import os
import sys
import functools

import numpy as np

for _p in ("/opt/trn_rl_repo", "/root/.axon_site/_ro/trn_rl_repo"):
    if os.path.isdir(_p) and _p not in sys.path:
        sys.path.insert(0, _p)

import ml_dtypes
import concourse.bass as bass
import concourse.mybir as mybir
import concourse.tile as tile
from concourse.vector_clock import ScopedClock
from concourse.masks import make_identity

BF16 = ml_dtypes.bfloat16
F32 = mybir.dt.float32
BF = mybir.dt.bfloat16

B, C, H, W = 8, 64, 128, 128
h2, w2 = 60, 60
PS = 6                      # patch size (hh == ww == 6)
KO = PS * PS                # 36 filter taps
KOP = 48                    # padded taps (xbar transpose needs mult of 16)
NPATCH = 100
SCALE = 1.0 / (PS * PS * NPATCH)   # 1/3600
NCORES = 8

HWPIX = H * W               # 16384
XHALF = HWPIX // 2          # 8192
PIX2 = h2 * w2              # 3600
X2CH = PIX2 // 8            # 450  (x2 matmul chunk)


class _PhaseStop(Exception):
    def __init__(self, ap):
        self.ap = ap


def _patch_tile_drain():
    """Split the TileContext tail-drain waits: this walrus build allows only
    ONE sync wait per instruction."""
    if getattr(tile.TileContext, "_drain_patched", False):
        return

    def _patched(self, tick_clock, wait_clock):
        nc = self.nc
        drain_inst = nc.sync.drain()
        wait_clock.add_sem_waits(
            drain_inst.ins, ScopedClock({None: tick_clock.global_clock})
        )
        si = drain_inst.ins.sync_info
        if si is not None and si.on_wait and len(si.on_wait) > 1:
            waits = list(si.on_wait)
            upd = list(si.on_update) if si.on_update else []
            drain_inst.ins.sync_info = mybir.SyncInfo(on_wait=waits[:1], on_update=upd)
            for w in waits[1:]:
                d2 = nc.sync.drain()
                d2.ins.sync_info = mybir.SyncInfo(on_wait=[w], on_update=[])
        nc.all_engine_barrier()
        popped = nc._tile_sem_poison_stack.pop()
        assert popped is self._sem_poison
        nc.clear_and_free_semaphores(list(self.sems.allocated().values()))
        nc.all_engine_barrier()

    tile.TileContext._drain_and_barrier = _patched
    tile.TileContext._drain_patched = True


def _split_excess_waits(nc):
    """Walrus here rejects >1 sync wait per instruction; move excess waits
    onto same-engine NoOps spliced immediately before the instruction."""
    n = 0
    for f in nc.m.functions:
        for bb in f.blocks:
            out = []
            for ins in bb.instructions:
                si = ins.sync_info
                if si is not None and si.on_wait and len(si.on_wait) > 1:
                    waits = list(si.on_wait)
                    for j, w in enumerate(waits[:-1]):
                        nop = mybir.InstNoOp(
                            name=f"{ins.name}_sw{j}",
                            engine=ins.engine,
                            ins=[],
                            outs=[],
                            sync_info=mybir.SyncInfo(on_wait=[w], on_update=[]),
                        )
                        out.append(nop)
                        n += 1
                    ins.sync_info = mybir.SyncInfo(
                        on_wait=[waits[-1]], on_update=list(si.on_update or [])
                    )
                out.append(ins)
            bb.instructions = out
    return n


def resize_weight_diagonals():
    """Replicate jax.image.resize(..., 'bilinear') 129->128 (antialias=True).

    Returns (w_lo[k], w_hi[k]) with out[i] = w_lo[i]*y[i] + w_hi[i]*y[i+1].
    """
    in_size, out_size = H + 1, H
    scale = out_size / in_size
    kernel_scale = max(1.0 / scale, 1.0)
    sample_f = ((np.arange(out_size, dtype=np.float32) + 0.5) / scale - 0.5)
    x = np.abs(sample_f[None, :] - np.arange(in_size, dtype=np.float32)[:, None])
    x = x / kernel_scale
    wmat = np.clip(1.0 - x, 0.0, None).astype(np.float32)  # [in, out]
    total = wmat.sum(axis=0, keepdims=True)
    wmat = np.where(np.abs(total) > 1e-6, wmat / total, 0.0).astype(np.float32)
    lo = np.array([wmat[i, i] for i in range(out_size)], np.float32)
    hi = np.array([wmat[i + 1, i] for i in range(out_size)], np.float32)
    chk = np.zeros_like(wmat)
    for i in range(out_size):
        chk[i, i] = lo[i]
        chk[i + 1, i] = hi[i]
    assert np.allclose(chk, wmat, atol=1e-6), "resize weights not 2-tap"
    return lo, hi


@functools.lru_cache(maxsize=4)
def build_program(repeats=1, split=True, phase=99):
    _patch_tile_drain()
    nc = bass.Bass()

    x1b = nc.dram_tensor("x1b", [128, XHALF], BF, kind="ExternalInput")
    x2c = nc.dram_tensor("x2c", [C, PIX2], BF, kind="ExternalInput")
    uv = nc.dram_tensor("uv", [4, 128, 128], F32, kind="ExternalInput")
    out = nc.dram_tensor("out", [128, 128], F32, kind="ExternalOutput")

    with tile.TileContext(nc) as tc:
        with tc.tile_pool(name="sb", bufs=1) as sb:
            for _rep in range(repeats):
                try:
                    # ---------- persistent SBUF ----------
                    sx1a = sb.tile([64, XHALF], BF)
                    nc.sync.dma_start(sx1a[:, :], x1b[0:64, :])
                    sx1b = sb.tile([64, XHALF], BF)
                    nc.sync.dma_start(sx1b[:, :], x1b[64:128, :])
                    sx1h = (sx1a, sx1b)
                    sUV = sb.tile([128, 4, 128], F32)
                    nc.sync.dma_start(sUV[:, :, :],
                                      uv[:, :, :].rearrange("q p i -> p q i"))
                    ident = sb.tile([128, 128], F32)
                    make_identity(nc, ident[:, :])
                    zero_b = sb.tile([64, 144], BF)
                    nc.vector.memset(zero_b[:, :], 0.0)

                    U0 = sUV[:, 0, :]
                    U1 = sUV[:, 1, :]
                    V0 = sUV[:, 2, :]
                    V1 = sUV[:, 3, :]

                    # ---------- x2 -> Kb ----------
                    Kb = sb.tile([64, KOP], BF)
                    nc.vector.memset(Kb[:, :], 0.0)
                    sb2_cm = tc.tile_pool(name="x2tmp", bufs=1)
                    sb2 = sb2_cm.__enter__()
                    PIX2P = 29 * 128                      # 3712 (pad to 128)
                    sx2 = sb2.tile([C, PIX2P], BF)
                    nc.vector.memset(sx2[:, :], 0.0)
                    nc.sync.dma_start(sx2[:, 0:PIX2], x2c[:, :])
                    x2sq = sb2.tile([C, PIX2P], BF)
                    nc.scalar.activation(x2sq[:, :], sx2[:, :],
                                         mybir.ActivationFunctionType.Square)
                    x2sqT = sb2.tile([128, 29, C], BF)
                    nc.sync.dma_start_transpose(x2sqT[:, :, :], x2sq[:, :])
                    s2T = sb2.tile([128, 29], F32)
                    nc.vector.tensor_reduce(s2T[:, :], x2sqT[:, :, :],
                                            axis=mybir.AxisListType.X,
                                            op=mybir.AluOpType.add)
                    s2s = sb2.tile([128, 29], F32)
                    nc.scalar.activation(s2s[:, :], s2T[:, :],
                                         mybir.ActivationFunctionType.Sqrt)
                    s2r = sb2.tile([128, 29], F32)
                    nc.vector.reciprocal(s2r[:, :], s2s[:, :])
                    dr_cm = tc.tile_pool(name="x2dram", bufs=1, space="DRAM")
                    dr = dr_cm.__enter__()
                    s2rd = dr.tile([128, 29], F32)
                    nc.sync.dma_start(s2rd[:, :], s2r[:, :])
                    # broadcast DRAM->SBUF replicated over the 64 channel
                    # partitions, kept in [c, p, blk] order (contiguous DMAs)
                    nrm2bcP = sb2.tile([C, 128, 29], F32)
                    bc_src = bass.AP(tensor=s2rd.tensor, offset=s2rd.offset,
                                     ap=[[0, C], [29, 128], [1, 29]])
                    nc.sync.dma_start(nrm2bcP[:, :, :], bc_src)
                    dr_cm.__exit__(None, None, None)
                    x2n = sb2.tile([C, PIX2], BF)
                    # full blocks: pix = blk*128 + p, blk in [0, 28)
                    in1a = bass.AP(tensor=nrm2bcP.tensor, offset=nrm2bcP.offset,
                                   ap=[[128 * 29, C], [1, 28], [29, 128]])
                    nc.vector.tensor_mul(
                        x2n[:, 0:3584].rearrange("c (b p) -> c b p", p=128),
                        sx2[:, 0:3584].rearrange("c (b p) -> c b p", p=128), in1a)
                    # tail block 28: pixels 3584..3599 (p in [0, 16))
                    in1b = bass.AP(tensor=nrm2bcP.tensor, offset=nrm2bcP.offset + 28,
                                   ap=[[128 * 29, C], [29, 16]])
                    nc.vector.tensor_mul(x2n[:, 3584:3600], sx2[:, 3584:3600], in1b)
                    # separable fold: sum over grid cells (i1 then i0)
                    a1 = sb2.tile([C, h2 * PS], F32)
                    in1 = bass.AP(tensor=x2n.tensor, offset=x2n.offset,
                                  ap=[[PIX2, C], [w2, h2], [1, PS], [PS, 10]])
                    nc.vector.tensor_reduce(a1[:, :], in1,
                                            axis=mybir.AxisListType.X,
                                            op=mybir.AluOpType.add)
                    Kf = sb2.tile([C, KO], F32)
                    in2 = bass.AP(tensor=a1.tensor, offset=a1.offset,
                                  ap=[[h2 * PS, C], [PS, PS], [1, PS], [KO, 10]])
                    nc.vector.tensor_reduce(Kf[:, :], in2,
                                            axis=mybir.AxisListType.X,
                                            op=mybir.AluOpType.add)
                    nc.vector.tensor_copy(Kb[:, 0:KO], Kf[:, :])
                    sb2_cm.__exit__(None, None, None)

                    if phase <= 1:
                        raise _PhaseStop(s2r)

                    # ---------- x1 pixel norms (transpose + free reduce) ----
                    sb3_cm = tc.tile_pool(name="x1tmp", bufs=1)
                    sb3 = sb3_cm.__enter__()
                    x1sqa = sb3.tile([64, XHALF], BF)
                    nc.scalar.activation(x1sqa[:, :], sx1a[:, :],
                                         mybir.ActivationFunctionType.Square)
                    x1sqb = sb3.tile([64, XHALF], BF)
                    nc.vector.tensor_mul(x1sqb[:, :], sx1b[:, :], sx1b[:, :])
                    # xbar: x1sqT[j, rr, c] = x1sq[c, rr*128 + j]
                    x1sqTa = sb3.tile([128, 64, C], BF)
                    nc.sync.dma_start_transpose(x1sqTa[:, :, :], x1sqa[:, :])
                    x1sqTb = sb3.tile([128, 64, C], BF)
                    nc.sync.dma_start_transpose(x1sqTb[:, :, :], x1sqb[:, :])
                    nsqT = sb.tile([128, 128], F32)       # [j, r]
                    nc.vector.tensor_reduce(nsqT[:, 0:64], x1sqTa[:, :, :],
                                            axis=mybir.AxisListType.X,
                                            op=mybir.AluOpType.add)
                    nc.vector.tensor_reduce(nsqT[:, 64:128], x1sqTb[:, :, :],
                                            axis=mybir.AxisListType.X,
                                            op=mybir.AluOpType.add)
                    sb3_cm.__exit__(None, None, None)
                    nsqs = sb.tile([128, 128], F32)
                    nc.scalar.activation(nsqs[:, :], nsqT[:, :],
                                         mybir.ActivationFunctionType.Sqrt)
                    nrmT = sb.tile([128, 128], F32)       # [j, r]
                    nc.vector.reciprocal(nrmT[:, :], nsqs[:, :])

                    if phase <= 2:
                        raise _PhaseStop(nrm)

                    # ---------- conv + transpose + normalize ----------
                    Zsb = sb.tile([KOP, HWPIX], BF)
                    with tc.tile_pool(name="psc", bufs=1, space="PSUM") as psc:
                        for t4 in range(4):
                            pz = psc.tile([KOP, 4096], F32, tag="zz")
                            for c4 in range(8):
                                k = t4 * 8 + c4
                                hh = k // 16
                                nn = k % 16
                                nc.tensor.matmul(
                                    pz[:, 512 * c4:512 * (c4 + 1)],
                                    Kb[:, :],
                                    sx1h[hh][:, 512 * nn:512 * (nn + 1)],
                                    start=True, stop=True)
                            nc.vector.tensor_copy(
                                Zsb[:, 4096 * t4:4096 * (t4 + 1)], pz[:, :])

                    if phase == 25:
                        dbg = sb.tile([KOP, 128], F32)
                        nc.vector.tensor_copy(dbg[:, :], Zsb[:, 0:128])
                        raise _PhaseStop(dbg)
                    # HW xbar semantics: ZTt[j, r, o] = Z[o, r*128 + j]
                    # (CoreSim models dma_start_transpose differently; the
                    # mapping here was verified on hardware.)
                    ZTt = sb.tile([128, 128, KOP], BF)
                    for q in range(4):
                        nc.sync.dma_start_transpose(
                            ZTt[:, 32 * q:32 * (q + 1), :],
                            Zsb[:, 4096 * q:4096 * (q + 1)])
                    ZT = sb.tile([128, KO, 128], F32)
                    zt_in = bass.AP(tensor=ZTt.tensor, offset=ZTt.offset,
                                    ap=[[128 * KOP, 128], [1, KO], [KOP, 128]])
                    nrm_bc = bass.AP(tensor=nrmT.tensor, offset=nrmT.offset,
                                     ap=[[128, 128], [0, KO], [1, 128]])
                    nc.vector.tensor_mul(ZT[:, :, :], zt_in, nrm_bc)

                    if phase <= 3:
                        raise _PhaseStop(ZT[:, 0, :])

                    # ---------- col2im stage 1: dy shifts (free dim) ----------
                    Ydx = sb.tile([128, PS, 133], F32)
                    nc.vector.memset(Ydx[:, :, :], 0.0)
                    for dy in range(PS):
                        sl = Ydx[:, :, 5 - dy:5 - dy + 128]
                        nc.vector.tensor_add(sl, sl, ZT[:, PS * dy:PS * dy + PS, :])

                    if phase <= 4:
                        raise _PhaseStop(Ydx[:, 0, 0:128])

                    # ---------- col2im stage 2: dx shifts via PE transposes ----
                    with tc.tile_pool(name="psb2", bufs=1, space="PSUM") as psb2:
                        YFp = psb2.tile([128, 144], F32)
                        nc.tensor.matmul(YFp[:, 0:135], zero_b[:, 0:128],
                                         zero_b[:, 0:135], start=True, stop=True)
                        for dx in range(PS):
                            nc.tensor.matmul(YFp[:, 6 - dx:6 - dx + 128],
                                             Ydx[:, dx, 2:130], ident[:, :],
                                             is_transpose=True, start=False,
                                             stop=(dx == PS - 1),
                                             skip_group_check=True)
                        YFxp = psb2.tile([1, 144], F32)
                        nc.tensor.matmul(YFxp[0:1, 0:135], zero_b[:, 0:1],
                                         zero_b[:, 0:135], start=True, stop=True)
                        for dx in range(PS):
                            nc.tensor.matmul(YFxp[0:1, 6 - dx:6 - dx + 128],
                                             Ydx[:, dx, 130:131], ident[:, :],
                                             is_transpose=True, start=False,
                                             stop=(dx == PS - 1),
                                             skip_group_check=True)
                        YFs = sb.tile([128, 135], F32)
                        nc.vector.tensor_copy(YFs[:, :], YFp[:, 0:135])
                        YFxs = sb.tile([1, 135], F32)
                        nc.vector.tensor_copy(YFxs[:, :], YFxp[0:1, 0:135])

                        if phase <= 5:
                            raise _PhaseStop(YFs[:, 0:128])

                        # ---------- resize stage B: J interp (free dim) -------
                        tmp0 = sb.tile([128, 128], F32)
                        tmp1 = sb.tile([128, 128], F32)
                        tJ = sb.tile([128, 128], F32)
                        nc.vector.tensor_mul(tmp0[:, :], YFs[:, 3:131], U0)
                        nc.vector.tensor_mul(tmp1[:, :], YFs[:, 4:132], U1)
                        nc.vector.tensor_add(tJ[:, :], tmp0[:, :], tmp1[:, :])
                        tJx = sb.tile([1, 128], F32)
                        nc.vector.tensor_mul(tmp0[0:1, :], YFxs[0:1, 3:131],
                                             U0[0:1, :])
                        nc.vector.tensor_mul(tmp1[0:1, :], YFxs[0:1, 4:132],
                                             U1[0:1, :])
                        nc.vector.tensor_add(tJx[:, :], tmp0[0:1, :], tmp1[0:1, :])

                        if phase <= 6:
                            raise _PhaseStop(tJ[:, 0:128])

                        # ---------- transpose to [J, I] ----------
                        ptJT = psb2.tile([128, 128], F32)
                        nc.tensor.transpose(ptJT[:, :], tJ[:, :], ident[:, :])
                        ptJx = psb2.tile([128, 1], F32)
                        nc.tensor.matmul(ptJx[:, :], tJx[0:1, :], ident[0:1, 0:1],
                                         is_transpose=True, start=True, stop=True)
                        tJT = sb.tile([128, 129], F32)
                        nc.vector.tensor_copy(tJT[:, 0:128], ptJT[:, :])
                        nc.vector.tensor_copy(tJT[:, 128:129], ptJx[:, :])

                        # ---------- resize stage C: I interp (free dim) -------
                        o0 = sb.tile([128, 128], F32)
                        o1t = sb.tile([128, 128], F32)
                        osb = sb.tile([128, 128], F32)
                        nc.vector.tensor_mul(o0[:, :], tJT[:, 0:128], V0)
                        nc.vector.tensor_mul(o1t[:, :], tJT[:, 1:129], V1)
                        nc.vector.tensor_add(osb[:, :], o0[:, :], o1t[:, :])
                        nc.sync.dma_start(out[:, :], osb[:, :])
                except _PhaseStop as e:
                    pp, ff = e.ap.shape[0], int(np.prod(e.ap.shape[1:]))
                    nc.sync.dma_start(out[0:pp, 0:ff], e.ap)

    if split:
        _split_excess_waits(nc)
    return nc


@functools.lru_cache(maxsize=1)
def _host_constants():
    lo, hi = resize_weight_diagonals()
    u0 = (lo * SCALE).astype(np.float32)
    u1 = (hi * SCALE).astype(np.float32)
    uv = np.stack([
        np.tile(u0[None, :], (128, 1)),
        np.tile(u1[None, :], (128, 1)),
        np.tile(lo[None, :], (128, 1)),
        np.tile(hi[None, :], (128, 1)),
    ]).astype(np.float32)
    return uv


def make_in_maps(x1, x2):
    x1 = np.asarray(x1)
    x2 = np.asarray(x2)
    uv = _host_constants()
    x1bf = x1.astype(BF16)
    in_maps = []
    for b in range(B):
        xb = x1bf[b]  # [64, 128, 128]
        x1arr = np.concatenate(
            [xb[:, :64].reshape(C, XHALF), xb[:, 64:].reshape(C, XHALF)], axis=0
        )  # [128, 8192]
        in_maps.append({
            "x1b": np.ascontiguousarray(x1arr),
            "x2c": np.ascontiguousarray(x2[b].reshape(C, PIX2).astype(BF16)),
            "uv": uv,
        })
    return in_maps


def kernel(x1, x2):
    from concourse.bass_utils import run_bass_kernel_spmd

    nc = build_program()
    in_maps = make_in_maps(x1, x2)
    res = run_bass_kernel_spmd(nc, in_maps, core_ids=list(range(NCORES)))
    outs = [np.asarray(res.results[b]["out"]).T for b in range(B)]  # [J,I]->[I,J]
    return np.stack(outs)[:, None].astype(np.float32)

